# revision 28
# baseline (speedup 1.0000x reference)
"""Trainium2 Bass kernel for nn_DiscreteStateSpaceModel_77077483094247.

Math: the reference computes y = einsum('nij,ijk->nik', u, K) but only uses
y[:, -1, :], so the whole model collapses to

    out = (u_t[:,-1,:] @ W_in.T + b_in) @ (C @ A_d^1023 @ B_d) @ W_out.T + b_out

A_d^1023 is built with binary exponentiation (9 squarings + 9 multiplies)
instead of the 1023-step serial scan.  Moreover A_d = expm(-0.01*HiPPO) is
lower triangular, so G = A_d^1023 is lower triangular with
G00 = (A_d[:128,:128])^1023 exactly; the coupling block G10 has norm ~2e-11
vs 5e-5 for G00 (validated on the actual inputs: dropping it moves the
output by <1e-7 of output scale), so the chain runs on 128x128 blocks and

    out = (w + 1 x bb1^T) @ G00 @ D + 1 x b_out
    w   = u_last @ W_in^T @ C[:, :128]          (chain-independent)
    bb1 = C[:, :128]^T @ b_in                   (chain-independent)
    D   = B_d[:128, :] @ W_out^T                (chain-independent)

Sharding: u_t is sharded over batch (2 rows per core); the small matrices are
replicated and the chain is duplicated per core (per the spec hint).

Scheduling: the serial 9-step chain (2 small matmuls + 2 PSUM->SBUF copies
per step) leaves PE bubbles; all chain-independent work (WC = W_in^T C,
w = u @ WC, D, bb1, 11 PE transposes) is emitted as filler jobs between
chain iterations so the PE stays dense and the HAM clock stays warm.  After
G00 only v = wb @ G00, a tiny transpose, and out = v^T^T @ D remain.

matmul computes lhsT.T @ rhs; fp32 everywhere (float32r's 11-bit mantissa
amplifies ~400x through the squaring chain - measured 9e-2 rel err).
"""

import numpy as np
from contextlib import ExitStack

from concourse import bacc, bass, mybir, tile
from concourse import bass_utils

B_SZ, SEQ, D_IN, H_DIM, D_OUT = 16, 1024, 512, 256, 512
N_CORES = 8
B_LOC = B_SZ // N_CORES  # 2 batch rows per core

F32 = mybir.dt.float32
P = 128  # partitions
NB = 128  # chain block size


def _build():
    nc = bacc.Bacc("TRN2", target_bir_lowering=False, debug=False,
                   num_devices=N_CORES)

    u_t = nc.dram_tensor("u_t", [B_LOC, SEQ, D_IN], F32, kind="ExternalInput")
    W_in = nc.dram_tensor("W_in", [H_DIM, D_IN], F32, kind="ExternalInput")
    b_in = nc.dram_tensor("b_in", [H_DIM], F32, kind="ExternalInput")
    C_t = nc.dram_tensor("C", [H_DIM, H_DIM], F32, kind="ExternalInput")
    W_out = nc.dram_tensor("W_out", [D_OUT, H_DIM], F32, kind="ExternalInput")
    b_out = nc.dram_tensor("b_out", [D_OUT], F32, kind="ExternalInput")
    A_d = nc.dram_tensor("A_d", [H_DIM, H_DIM], F32, kind="ExternalInput")
    B_d = nc.dram_tensor("B_d", [H_DIM, H_DIM], F32, kind="ExternalInput")
    eye = nc.dram_tensor("eye", [P, P], F32, kind="ExternalInput")
    A00T = nc.dram_tensor("a00t", [NB, NB], F32, kind="ExternalInput")
    ones2 = nc.dram_tensor("ones2", [1, B_LOC], F32, kind="ExternalInput")
    out = nc.dram_tensor("out", [B_LOC, D_OUT], F32, kind="ExternalOutput")

    with tile.TileContext(nc) as tc, ExitStack() as ctx:
        const = ctx.enter_context(tc.tile_pool(name="const", bufs=1))
        work = ctx.enter_context(tc.tile_pool(name="work", bufs=2))
        psum = ctx.enter_context(
            tc.tile_pool(name="psum", bufs=2, space=bass.MemorySpace.PSUM))

        V = nc.vector
        MM = nc.tensor.matmul

        # ---- loads; the chain needs a00 + eye immediately -------------------
        a00_sb = const.tile([NB, NB], F32, tag="a00")
        nc.sync.dma_start(a00_sb[:], A_d.ap()[0:NB, 0:NB])
        a00t_sb = const.tile([NB, NB], F32, tag="a00t")
        nc.scalar.dma_start(a00t_sb[:], A00T.ap()[:, :])
        id_sb = const.tile([P, P], F32, tag="id")
        nc.scalar.dma_start(id_sb[:], eye.ap()[:, :])
        c_sb = const.tile([P, 2, H_DIM], F32, tag="c")
        nc.scalar.dma_start(c_sb[:], C_t.ap().rearrange("(b p) d -> p b d", p=P))

        btop_sb = const.tile([NB, H_DIM], F32, tag="btop")
        nc.sync.dma_start(btop_sb[:], B_d.ap()[0:NB, :])
        wo_sb = const.tile([P, 4, H_DIM], F32, tag="wo")
        nc.sync.dma_start(wo_sb[:], W_out.ap().rearrange("(b p) d -> p b d", p=P))

        wi_sb = const.tile([P, 2, D_IN], F32, tag="wi")
        nc.scalar.dma_start(wi_sb[:], W_in.ap().rearrange("(b p) d -> p b d", p=P))
        bin_sb = const.tile([P, 2], F32, tag="bin")
        nc.scalar.dma_start(bin_sb[:], b_in.ap().rearrange("(b p) -> p b", p=P))
        bout_sb = const.tile([1, D_OUT], F32, tag="bout")
        nc.scalar.dma_start(bout_sb[:], b_out.ap()[None, :])
        ones2_sb = const.tile([1, B_LOC], F32, tag="ones2")
        nc.scalar.dma_start(ones2_sb[:], ones2.ap()[:, :])

        # u_last^T: [512, 2] column layout -> sbuf [128, 4ko, 2]
        ult_sb = const.tile([P, 4, B_LOC], F32, tag="ult")
        for n in range(B_LOC):
            nc.gpsimd.dma_start(
                ult_sb[:, :, n:n + 1],
                u_t.ap()[n, SEQ - 1, :].rearrange("(k p) -> p k", p=P)[:, :, None])

        # ---- chain-independent work as filler jobs --------------------------
        wot_sb = const.tile([P, 2, D_OUT], F32, tag="wot")
        bt_sb = const.tile([P, 2, NB], F32, tag="bt")
        wc_sb = work.tile([P, 4, NB], F32, tag="wc")
        w_sb = work.tile([B_LOC, NB], F32, tag="w")
        d_sb = work.tile([NB, D_OUT], F32, tag="d")
        bb1_sb = work.tile([NB, 2], F32, tag="bb")  # bb1 duplicated in 2 cols
        wbt_sb = work.tile([NB, B_LOC], F32, tag="wbt")

        S = nc.scalar

        def tr_job(dst, dst_sl, src, src_sl, np_, idp):
            def go():
                tps = psum.tile([np_, idp], F32, tag="sm_tr", bufs=3)
                nc.tensor.transpose(tps[:], src[src_sl], id_sb[0:idp, 0:idp])
                V.tensor_copy(dst[dst_sl], tps[:])
            return go

        def wc_job(mo):
            # WC = W_in^T @ C[:, :NB]   [512, NB], mo-block
            def go():
                ps = psum.tile([P, NB], F32, tag="sm_tr", bufs=3)
                for ko in range(2):
                    MM(ps[:], wi_sb[:, ko, P * mo:P * (mo + 1)],
                       c_sb[:, ko, 0:NB], start=(ko == 0), stop=(ko == 1))
                V.tensor_copy(wc_sb[:, mo, :], ps[:])
            return go

        def w_job():
            # w = u_last @ WC            [2, NB]
            ps = psum.tile([B_LOC, NB], F32, tag="sm_st", bufs=1)
            for ko in range(4):
                MM(ps[:], ult_sb[:, ko, :], wc_sb[:, ko, :],
                   start=(ko == 0), stop=(ko == 3))
            V.tensor_copy(w_sb[:], ps[:])

        def bb1_job():
            # bb1 = C[:, :NB]^T @ b_in  [NB, 1], stored twice
            ps = psum.tile([NB, 1], F32, tag="sm_st", bufs=1)
            for ko in range(2):
                MM(ps[:], c_sb[:, ko, 0:NB], bin_sb[:, ko:ko + 1],
                   start=(ko == 0), stop=(ko == 1))
            V.tensor_copy(bb1_sb[:, 0:1], ps[:])
            V.tensor_copy(bb1_sb[:, 1:2], ps[:])

        def wbt_job():
            # wb^T = w^T + bb1 (bias broadcast over the 2 batch cols) [NB, 2]
            tps = psum.tile([NB, B_LOC], F32, tag="sm_tr", bufs=3)
            nc.tensor.transpose(tps[:], w_sb[:], id_sb[0:B_LOC, 0:B_LOC])
            V.tensor_tensor(wbt_sb[:], tps[:], bb1_sb[:],
                            op=mybir.AluOpType.add)

        ps_out = psum.tile([B_LOC, D_OUT], F32, tag="big")

        def bias_seed_job():
            # out-psum starts as 1 x b_out; the final vt@D accumulates on top
            MM(ps_out[:], ones2_sb[:], bout_sb[:], start=True, stop=False)

        def d_job():
            # D = Btop @ W_out^T         [NB, 512]
            ps = psum.tile([NB, D_OUT], F32, tag="big")
            for ko in range(2):
                MM(ps[:], bt_sb[:, ko, :], wot_sb[:, ko, :],
                   start=(ko == 0), stop=(ko == 1))
            V.tensor_copy(d_sb[:], ps[:])

        jobs = [tr_job(bt_sb, np.s_[:, c, :], btop_sb,
                       np.s_[:, P * c:P * (c + 1)], P, NB)
                for c in range(2)]
        jobs += [tr_job(wot_sb, np.s_[:, r, P * c:P * (c + 1)],
                        wo_sb, np.s_[:, c, P * r:P * (r + 1)], P, P)
                 for r in range(2) for c in range(4)]
        jobs += [wc_job(mo) for mo in range(4)]
        jobs += [bb1_job, bias_seed_job, w_job, d_job, wbt_job]

        def emit_jobs(n):
            for _ in range(n):
                if jobs:
                    jobs.pop(0)()

        # ---- 128x128 power chain: XA = [S | Pacc], st = S^T -----------------
        # xast = [S | Pacc | S^T] in one tile; both per-iter matmuls write
        # one PSUM bank so each iteration needs a single copy + sem wait.
        # Iter 1 runs straight off (a00, a00t) with Pacc_1 = A00, so after
        # iters 2..9 apply factors A^2..A^256, Pacc_9 = A^511 and
        # G00 = S_9 @ Pacc_9 = A^512 @ A^511 = A^1023.
        xa_cur = work.tile([NB, 3 * NB], F32, tag="xa")
        ps = psum.tile([NB, 4 * NB], F32, tag="sm_xa")
        MM(ps[:, 0:NB], a00t_sb[:], a00_sb[:], start=True, stop=True)
        MM(ps[:, 2 * NB:3 * NB], a00_sb[:], a00t_sb[:], start=True, stop=True)
        V.tensor_copy(xa_cur[:, 0:NB], ps[:, 0:NB])
        V.tensor_copy(xa_cur[:, 2 * NB:3 * NB], ps[:, 2 * NB:3 * NB])
        V.tensor_copy(xa_cur[:, NB:2 * NB], a00_sb[:])

        for k in range(2, 10):
            xa_new = work.tile([NB, 3 * NB], F32, tag="xa")
            ps = psum.tile([NB, 4 * NB], F32, tag="sm_xa")
            MM(ps[:, 0:2 * NB], xa_cur[:, 2 * NB:3 * NB], xa_cur[:, 0:2 * NB],
               start=True, stop=True)
            MM(ps[:, 2 * NB:3 * NB], xa_cur[:, 0:NB], xa_cur[:, 2 * NB:3 * NB],
               start=True, stop=True)
            V.tensor_copy(xa_new[:], ps[:, 0:3 * NB])
            xa_cur = xa_new
            emit_jobs([1, 2, 2, 3, 3, 3, 3, 2][k - 2])
        emit_jobs(len(jobs))

        # ---- post-chain tail --------------------------------------------
        # v^T = G00^T wb^T = p9^T (s9^T wb^T);  out = v^T^T @ D + bias-seed
        z_sb = work.tile([NB, B_LOC], F32, tag="z")
        ps = psum.tile([NB, B_LOC], F32, tag="sm_st", bufs=1)
        MM(ps[:], xa_cur[:, 0:NB], wbt_sb[:], start=True, stop=True)
        V.tensor_copy(z_sb[:], ps[:])

        vt_sb = work.tile([NB, B_LOC], F32, tag="vt")
        ps = psum.tile([NB, B_LOC], F32, tag="sm_tr", bufs=3)
        MM(ps[:], xa_cur[:, NB:2 * NB], z_sb[:], start=True, stop=True)
        V.tensor_copy(vt_sb[:], ps[:])

        MM(ps_out[:], vt_sb[:], d_sb[:], start=False, stop=True)
        out_sb = work.tile([B_LOC, D_OUT], F32, tag="osb")
        V.tensor_copy(out_sb[:], ps_out[:])
        nc.sync.dma_start(out.ap()[:, :], out_sb[:])

    nc.compile()
    return nc


_NC_CACHE = {}


def _get_nc():
    if "nc" not in _NC_CACHE:
        _NC_CACHE["nc"] = _build()
    return _NC_CACHE["nc"]


_EYE = np.eye(P, dtype=np.float32)
_ONES2 = np.ones((1, B_LOC), dtype=np.float32)


def kernel(u_t, W_in, b_in, C, W_out, b_out, A_d, B_d, **run_kwargs):
    nc = _get_nc()
    u_t = np.ascontiguousarray(u_t, dtype=np.float32)
    shared = {
        "W_in": np.ascontiguousarray(W_in, dtype=np.float32),
        "b_in": np.ascontiguousarray(b_in, dtype=np.float32),
        "C": np.ascontiguousarray(C, dtype=np.float32),
        "W_out": np.ascontiguousarray(W_out, dtype=np.float32),
        "b_out": np.ascontiguousarray(b_out, dtype=np.float32),
        "A_d": np.ascontiguousarray(A_d, dtype=np.float32),
        "B_d": np.ascontiguousarray(B_d, dtype=np.float32),
        "eye": _EYE,
        "a00t": np.ascontiguousarray(np.asarray(A_d, dtype=np.float32)[0:NB, 0:NB].T),
        "ones2": _ONES2,
    }
    in_maps = []
    for i in range(N_CORES):
        m = dict(shared)
        m["u_t"] = np.ascontiguousarray(u_t[i * B_LOC:(i + 1) * B_LOC])
        in_maps.append(m)
    res = bass_utils.run_bass_kernel_spmd(
        nc, in_maps, core_ids=list(range(N_CORES)), **run_kwargs)
    out = np.concatenate([res.results[i]["out"] for i in range(N_CORES)], axis=0)
    if run_kwargs:
        return out, res
    return out


# revision 29
# speedup vs baseline: 1.0135x; 1.0135x over previous
"""Trainium2 Bass kernel for nn_DiscreteStateSpaceModel_77077483094247.

Math: the reference computes y = einsum('nij,ijk->nik', u, K) but only uses
y[:, -1, :], so the whole model collapses to

    out = (u_t[:,-1,:] @ W_in.T + b_in) @ (C @ A_d^1023 @ B_d) @ W_out.T + b_out

A_d^1023 is built with binary exponentiation (9 squarings + 9 multiplies)
instead of the 1023-step serial scan.  Moreover A_d = expm(-0.01*HiPPO) is
lower triangular, so G = A_d^1023 is lower triangular with
G00 = (A_d[:128,:128])^1023 exactly; the coupling block G10 has norm ~2e-11
vs 5e-5 for G00 (validated on the actual inputs: dropping it moves the
output by <1e-7 of output scale), so the chain runs on 128x128 blocks and

    out = (w + 1 x bb1^T) @ G00 @ D + 1 x b_out
    w   = u_last @ W_in^T @ C[:, :128]          (chain-independent)
    bb1 = C[:, :128]^T @ b_in                   (chain-independent)
    D   = B_d[:128, :] @ W_out^T                (chain-independent)

Sharding: u_t is sharded over batch (2 rows per core); the small matrices are
replicated and the chain is duplicated per core (per the spec hint).

Scheduling: the serial 9-step chain (2 small matmuls + 2 PSUM->SBUF copies
per step) leaves PE bubbles; all chain-independent work (WC = W_in^T C,
w = u @ WC, D, bb1, 11 PE transposes) is emitted as filler jobs between
chain iterations so the PE stays dense and the HAM clock stays warm.  After
G00 only v = wb @ G00, a tiny transpose, and out = v^T^T @ D remain.

matmul computes lhsT.T @ rhs; fp32 everywhere (float32r's 11-bit mantissa
amplifies ~400x through the squaring chain - measured 9e-2 rel err).
"""

import numpy as np
from contextlib import ExitStack

from concourse import bacc, bass, mybir, tile
from concourse import bass_utils

B_SZ, SEQ, D_IN, H_DIM, D_OUT = 16, 1024, 512, 256, 512
N_CORES = 8
B_LOC = B_SZ // N_CORES  # 2 batch rows per core

F32 = mybir.dt.float32
P = 128  # partitions
NB = 128  # chain block size


def _build():
    nc = bacc.Bacc("TRN2", target_bir_lowering=False, debug=False,
                   num_devices=N_CORES)

    u_t = nc.dram_tensor("u_t", [B_LOC, SEQ, D_IN], F32, kind="ExternalInput")
    W_in = nc.dram_tensor("W_in", [H_DIM, D_IN], F32, kind="ExternalInput")
    b_in = nc.dram_tensor("b_in", [H_DIM], F32, kind="ExternalInput")
    C_t = nc.dram_tensor("C", [H_DIM, H_DIM], F32, kind="ExternalInput")
    W_out = nc.dram_tensor("W_out", [D_OUT, H_DIM], F32, kind="ExternalInput")
    b_out = nc.dram_tensor("b_out", [D_OUT], F32, kind="ExternalInput")
    A_d = nc.dram_tensor("A_d", [H_DIM, H_DIM], F32, kind="ExternalInput")
    B_d = nc.dram_tensor("B_d", [H_DIM, H_DIM], F32, kind="ExternalInput")
    eye = nc.dram_tensor("eye", [P, P], F32, kind="ExternalInput")
    A00T = nc.dram_tensor("a00t", [NB, NB], F32, kind="ExternalInput")
    ones2 = nc.dram_tensor("ones2", [1, B_LOC], F32, kind="ExternalInput")
    out = nc.dram_tensor("out", [B_LOC, D_OUT], F32, kind="ExternalOutput")

    with tile.TileContext(nc) as tc, ExitStack() as ctx:
        const = ctx.enter_context(tc.tile_pool(name="const", bufs=1))
        work = ctx.enter_context(tc.tile_pool(name="work", bufs=2))
        psum = ctx.enter_context(
            tc.tile_pool(name="psum", bufs=2, space=bass.MemorySpace.PSUM))

        V = nc.vector
        MM = nc.tensor.matmul

        # ---- loads; the chain needs a00 + eye immediately -------------------
        a00_sb = const.tile([NB, NB], F32, tag="a00")
        nc.sync.dma_start(a00_sb[:], A_d.ap()[0:NB, 0:NB])
        a00t_sb = const.tile([NB, NB], F32, tag="a00t")
        nc.scalar.dma_start(a00t_sb[:], A00T.ap()[:, :])
        id_sb = const.tile([P, P], F32, tag="id")
        nc.scalar.dma_start(id_sb[:], eye.ap()[:, :])
        c_sb = const.tile([P, 2, H_DIM], F32, tag="c")
        nc.scalar.dma_start(c_sb[:], C_t.ap().rearrange("(b p) d -> p b d", p=P))

        btop_sb = const.tile([NB, H_DIM], F32, tag="btop")
        nc.sync.dma_start(btop_sb[:], B_d.ap()[0:NB, :])
        wo_sb = const.tile([P, 4, H_DIM], F32, tag="wo")
        nc.sync.dma_start(wo_sb[:], W_out.ap().rearrange("(b p) d -> p b d", p=P))

        wi_sb = const.tile([P, 2, D_IN], F32, tag="wi")
        nc.scalar.dma_start(wi_sb[:], W_in.ap().rearrange("(b p) d -> p b d", p=P))
        bin_sb = const.tile([P, 2], F32, tag="bin")
        nc.scalar.dma_start(bin_sb[:], b_in.ap().rearrange("(b p) -> p b", p=P))
        bout_sb = const.tile([1, D_OUT], F32, tag="bout")
        nc.scalar.dma_start(bout_sb[:], b_out.ap()[None, :])
        ones2_sb = const.tile([1, B_LOC], F32, tag="ones2")
        nc.scalar.dma_start(ones2_sb[:], ones2.ap()[:, :])

        # u_last^T: [512, 2] column layout -> sbuf [128, 4ko, 2]
        ult_sb = const.tile([P, 4, B_LOC], F32, tag="ult")
        for n in range(B_LOC):
            nc.gpsimd.dma_start(
                ult_sb[:, :, n:n + 1],
                u_t.ap()[n, SEQ - 1, :].rearrange("(k p) -> p k", p=P)[:, :, None])

        # ---- chain-independent work as filler jobs --------------------------
        wot_sb = const.tile([P, 2, D_OUT], F32, tag="wot")
        bt_sb = const.tile([P, 2, NB], F32, tag="bt")
        wc_sb = work.tile([P, 4, NB], F32, tag="wc")
        w_sb = work.tile([B_LOC, NB], F32, tag="w")
        d_sb = work.tile([NB, D_OUT], F32, tag="d")
        bb1_sb = work.tile([NB, 2], F32, tag="bb")  # bb1 duplicated in 2 cols
        wbt_sb = work.tile([NB, B_LOC], F32, tag="wbt")

        S = nc.scalar

        def tr_job(dst, dst_sl, src, src_sl, np_, idp):
            def go():
                tps = psum.tile([np_, idp], F32, tag="sm_tr", bufs=3)
                nc.tensor.transpose(tps[:], src[src_sl], id_sb[0:idp, 0:idp])
                V.tensor_copy(dst[dst_sl], tps[:])
            return go

        def wc_job(mo):
            # WC = W_in^T @ C[:, :NB]   [512, NB], mo-block
            def go():
                ps = psum.tile([P, NB], F32, tag="sm_tr", bufs=3)
                for ko in range(2):
                    MM(ps[:], wi_sb[:, ko, P * mo:P * (mo + 1)],
                       c_sb[:, ko, 0:NB], start=(ko == 0), stop=(ko == 1))
                V.tensor_copy(wc_sb[:, mo, :], ps[:])
            return go

        def w_job():
            # w = u_last @ WC            [2, NB]
            ps = psum.tile([B_LOC, NB], F32, tag="sm_st", bufs=1)
            for ko in range(4):
                MM(ps[:], ult_sb[:, ko, :], wc_sb[:, ko, :],
                   start=(ko == 0), stop=(ko == 3))
            V.tensor_copy(w_sb[:], ps[:])

        def bb1_job():
            # bb1 = C[:, :NB]^T @ b_in  [NB, 1], stored twice
            ps = psum.tile([NB, 1], F32, tag="sm_st", bufs=1)
            for ko in range(2):
                MM(ps[:], c_sb[:, ko, 0:NB], bin_sb[:, ko:ko + 1],
                   start=(ko == 0), stop=(ko == 1))
            V.tensor_copy(bb1_sb[:, 0:1], ps[:])
            V.tensor_copy(bb1_sb[:, 1:2], ps[:])

        def wbt_job():
            # wb^T = w^T + bb1 (bias broadcast over the 2 batch cols) [NB, 2]
            tps = psum.tile([NB, B_LOC], F32, tag="sm_tr", bufs=3)
            nc.tensor.transpose(tps[:], w_sb[:], id_sb[0:B_LOC, 0:B_LOC])
            V.tensor_tensor(wbt_sb[:], tps[:], bb1_sb[:],
                            op=mybir.AluOpType.add)

        ps_out = psum.tile([B_LOC, D_OUT], F32, tag="big")

        def bias_seed_job():
            # out-psum starts as 1 x b_out; the final vt@D accumulates on top
            MM(ps_out[:], ones2_sb[:], bout_sb[:], start=True, stop=False)

        def d_job():
            # D = Btop @ W_out^T         [NB, 512]
            ps = psum.tile([NB, D_OUT], F32, tag="big")
            for ko in range(2):
                MM(ps[:], bt_sb[:, ko, :], wot_sb[:, ko, :],
                   start=(ko == 0), stop=(ko == 1))
            V.tensor_copy(d_sb[:], ps[:])

        jobs = [tr_job(bt_sb, np.s_[:, c, :], btop_sb,
                       np.s_[:, P * c:P * (c + 1)], P, NB)
                for c in range(2)]
        jobs += [tr_job(wot_sb, np.s_[:, r, P * c:P * (c + 1)],
                        wo_sb, np.s_[:, c, P * r:P * (r + 1)], P, P)
                 for r in range(2) for c in range(4)]
        jobs += [wc_job(mo) for mo in range(4)]
        jobs += [bb1_job, bias_seed_job, w_job, d_job, wbt_job]

        def emit_jobs(n):
            for _ in range(n):
                if jobs:
                    jobs.pop(0)()

        # ---- 128x128 power chain: XA = [S | Pacc], st = S^T -----------------
        # xast = [S | Pacc | S^T] in one tile; both per-iter matmuls write
        # one PSUM bank so each iteration needs a single copy + sem wait.
        # Iter 1 runs straight off (a00, a00t) with Pacc_1 = A00, so after
        # iters 2..9 apply factors A^2..A^256, Pacc_9 = A^511 and
        # G00 = S_9 @ Pacc_9 = A^512 @ A^511 = A^1023.
        xa_cur = work.tile([NB, 3 * NB], F32, tag="xa")
        ps = psum.tile([NB, 4 * NB], F32, tag="sm_xa")
        MM(ps[:, 0:NB], a00t_sb[:], a00_sb[:], start=True, stop=True)
        MM(ps[:, 2 * NB:3 * NB], a00_sb[:], a00t_sb[:], start=True, stop=True)
        V.tensor_copy(xa_cur[:, 0:NB], ps[:, 0:NB])
        V.tensor_copy(xa_cur[:, 2 * NB:3 * NB], ps[:, 2 * NB:3 * NB])
        V.tensor_copy(xa_cur[:, NB:2 * NB], a00_sb[:])

        for k in range(2, 10):
            xa_new = work.tile([NB, 3 * NB], F32, tag="xa")
            ps = psum.tile([NB, 4 * NB], F32, tag="sm_xa")
            MM(ps[:, 0:2 * NB], xa_cur[:, 2 * NB:3 * NB], xa_cur[:, 0:2 * NB],
               start=True, stop=True)
            MM(ps[:, 2 * NB:3 * NB], xa_cur[:, 0:NB], xa_cur[:, 2 * NB:3 * NB],
               start=True, stop=True)
            V.tensor_copy(xa_new[:], ps[:, 0:3 * NB])
            xa_cur = xa_new
            emit_jobs([1, 2, 2, 3, 3, 3, 3, 2][k - 2])
        emit_jobs(len(jobs))

        # ---- post-chain tail --------------------------------------------
        # v^T = G00^T wb^T = p9^T (s9^T wb^T);  out = v^T^T @ D + bias-seed
        z_sb = work.tile([NB, B_LOC], F32, tag="z")
        ps = psum.tile([NB, B_LOC], F32, tag="sm_st", bufs=1)
        MM(ps[:], xa_cur[:, 0:NB], wbt_sb[:], start=True, stop=True)
        V.tensor_copy(z_sb[:], ps[:])

        vt_sb = work.tile([NB, B_LOC], F32, tag="vt")
        ps = psum.tile([NB, B_LOC], F32, tag="sm_tr", bufs=3)
        MM(ps[:], xa_cur[:, NB:2 * NB], z_sb[:], start=True, stop=True)
        V.tensor_copy(vt_sb[:], ps[:])

        # split the output tail into halves so half-0's copy+DMA overlaps
        # half-1's matmul; the two DMAs go out on different HWDGE queues
        H2 = D_OUT // 2
        out_sb = work.tile([B_LOC, D_OUT], F32, tag="osb")
        MM(ps_out[:, 0:H2], vt_sb[:], d_sb[:, 0:H2], start=False, stop=True)
        V.tensor_copy(out_sb[:, 0:H2], ps_out[:, 0:H2])
        nc.sync.dma_start(out.ap()[:, 0:H2], out_sb[:, 0:H2])
        MM(ps_out[:, H2:D_OUT], vt_sb[:], d_sb[:, H2:D_OUT],
           start=False, stop=True)
        V.tensor_copy(out_sb[:, H2:D_OUT], ps_out[:, H2:D_OUT])
        nc.scalar.dma_start(out.ap()[:, H2:D_OUT], out_sb[:, H2:D_OUT])

    nc.compile()
    return nc


_NC_CACHE = {}


def _get_nc():
    if "nc" not in _NC_CACHE:
        _NC_CACHE["nc"] = _build()
    return _NC_CACHE["nc"]


_EYE = np.eye(P, dtype=np.float32)
_ONES2 = np.ones((1, B_LOC), dtype=np.float32)


def kernel(u_t, W_in, b_in, C, W_out, b_out, A_d, B_d, **run_kwargs):
    nc = _get_nc()
    u_t = np.ascontiguousarray(u_t, dtype=np.float32)
    shared = {
        "W_in": np.ascontiguousarray(W_in, dtype=np.float32),
        "b_in": np.ascontiguousarray(b_in, dtype=np.float32),
        "C": np.ascontiguousarray(C, dtype=np.float32),
        "W_out": np.ascontiguousarray(W_out, dtype=np.float32),
        "b_out": np.ascontiguousarray(b_out, dtype=np.float32),
        "A_d": np.ascontiguousarray(A_d, dtype=np.float32),
        "B_d": np.ascontiguousarray(B_d, dtype=np.float32),
        "eye": _EYE,
        "a00t": np.ascontiguousarray(np.asarray(A_d, dtype=np.float32)[0:NB, 0:NB].T),
        "ones2": _ONES2,
    }
    in_maps = []
    for i in range(N_CORES):
        m = dict(shared)
        m["u_t"] = np.ascontiguousarray(u_t[i * B_LOC:(i + 1) * B_LOC])
        in_maps.append(m)
    res = bass_utils.run_bass_kernel_spmd(
        nc, in_maps, core_ids=list(range(N_CORES)), **run_kwargs)
    out = np.concatenate([res.results[i]["out"] for i in range(N_CORES)], axis=0)
    if run_kwargs:
        return out, res
    return out


# revision 30
# speedup vs baseline: 1.0207x; 1.0072x over previous
"""Trainium2 Bass kernel for nn_DiscreteStateSpaceModel_77077483094247.

Math: the reference computes y = einsum('nij,ijk->nik', u, K) but only uses
y[:, -1, :], so the whole model collapses to

    out = (u_t[:,-1,:] @ W_in.T + b_in) @ (C @ A_d^1023 @ B_d) @ W_out.T + b_out

A_d^1023 is built with binary exponentiation (9 squarings + 9 multiplies)
instead of the 1023-step serial scan.  Moreover A_d = expm(-0.01*HiPPO) is
lower triangular, so G = A_d^1023 is lower triangular with
G00 = (A_d[:128,:128])^1023 exactly; the coupling block G10 has norm ~2e-11
vs 5e-5 for G00 (validated on the actual inputs: dropping it moves the
output by <1e-7 of output scale), so the chain runs on 128x128 blocks and

    out = (w + 1 x bb1^T) @ G00 @ D + 1 x b_out
    w   = u_last @ W_in^T @ C[:, :128]          (chain-independent)
    bb1 = C[:, :128]^T @ b_in                   (chain-independent)
    D   = B_d[:128, :] @ W_out^T                (chain-independent)

Sharding: u_t is sharded over batch (2 rows per core); the small matrices are
replicated and the chain is duplicated per core (per the spec hint).

Scheduling: the serial 9-step chain (2 small matmuls + 2 PSUM->SBUF copies
per step) leaves PE bubbles; all chain-independent work (WC = W_in^T C,
w = u @ WC, D, bb1, 11 PE transposes) is emitted as filler jobs between
chain iterations so the PE stays dense and the HAM clock stays warm.  After
G00 only v = wb @ G00, a tiny transpose, and out = v^T^T @ D remain.

matmul computes lhsT.T @ rhs; fp32 everywhere (float32r's 11-bit mantissa
amplifies ~400x through the squaring chain - measured 9e-2 rel err).
"""

import numpy as np
from contextlib import ExitStack

from concourse import bacc, bass, mybir, tile
from concourse import bass_utils

B_SZ, SEQ, D_IN, H_DIM, D_OUT = 16, 1024, 512, 256, 512
N_CORES = 8
B_LOC = B_SZ // N_CORES  # 2 batch rows per core

F32 = mybir.dt.float32
P = 128  # partitions
NB = 128  # chain block size


def _build():
    nc = bacc.Bacc("TRN2", target_bir_lowering=False, debug=False,
                   num_devices=N_CORES)

    u_t = nc.dram_tensor("u_t", [B_LOC, SEQ, D_IN], F32, kind="ExternalInput")
    W_in = nc.dram_tensor("W_in", [H_DIM, D_IN], F32, kind="ExternalInput")
    b_in = nc.dram_tensor("b_in", [H_DIM], F32, kind="ExternalInput")
    C_t = nc.dram_tensor("C", [H_DIM, H_DIM], F32, kind="ExternalInput")
    W_out = nc.dram_tensor("W_out", [D_OUT, H_DIM], F32, kind="ExternalInput")
    b_out = nc.dram_tensor("b_out", [D_OUT], F32, kind="ExternalInput")
    A_d = nc.dram_tensor("A_d", [H_DIM, H_DIM], F32, kind="ExternalInput")
    B_d = nc.dram_tensor("B_d", [H_DIM, H_DIM], F32, kind="ExternalInput")
    eye = nc.dram_tensor("eye", [P, P], F32, kind="ExternalInput")
    A00T = nc.dram_tensor("a00t", [NB, NB], F32, kind="ExternalInput")
    ones2 = nc.dram_tensor("ones2", [1, B_LOC], F32, kind="ExternalInput")
    out = nc.dram_tensor("out", [B_LOC, D_OUT], F32, kind="ExternalOutput")

    with tile.TileContext(nc) as tc, ExitStack() as ctx:
        const = ctx.enter_context(tc.tile_pool(name="const", bufs=1))
        work = ctx.enter_context(tc.tile_pool(name="work", bufs=2))
        psum = ctx.enter_context(
            tc.tile_pool(name="psum", bufs=2, space=bass.MemorySpace.PSUM))

        V = nc.vector
        MM = nc.tensor.matmul

        # ---- loads; the chain needs a00 + eye immediately -------------------
        a00_sb = const.tile([NB, NB], F32, tag="a00")
        nc.sync.dma_start(a00_sb[:], A_d.ap()[0:NB, 0:NB])
        a00t_sb = const.tile([NB, NB], F32, tag="a00t")
        nc.scalar.dma_start(a00t_sb[:], A00T.ap()[:, :])
        id_sb = const.tile([P, P], F32, tag="id")
        nc.scalar.dma_start(id_sb[:], eye.ap()[:, :])
        c_sb = const.tile([P, 2, H_DIM], F32, tag="c")
        nc.scalar.dma_start(c_sb[:], C_t.ap().rearrange("(b p) d -> p b d", p=P))

        btop_sb = const.tile([NB, H_DIM], F32, tag="btop")
        nc.sync.dma_start(btop_sb[:], B_d.ap()[0:NB, :])
        wo_sb = const.tile([P, 4, H_DIM], F32, tag="wo")
        nc.sync.dma_start(wo_sb[:], W_out.ap().rearrange("(b p) d -> p b d", p=P))

        wi_sb = const.tile([P, 2, D_IN], F32, tag="wi")
        nc.scalar.dma_start(wi_sb[:], W_in.ap().rearrange("(b p) d -> p b d", p=P))
        bin_sb = const.tile([P, 2], F32, tag="bin")
        nc.scalar.dma_start(bin_sb[:], b_in.ap().rearrange("(b p) -> p b", p=P))
        bout_sb = const.tile([1, D_OUT], F32, tag="bout")
        nc.scalar.dma_start(bout_sb[:], b_out.ap()[None, :])
        ones2_sb = const.tile([1, B_LOC], F32, tag="ones2")
        nc.scalar.dma_start(ones2_sb[:], ones2.ap()[:, :])

        # u_last^T: [512, 2] column layout -> sbuf [128, 4ko, 2]
        ult_sb = const.tile([P, 4, B_LOC], F32, tag="ult")
        for n in range(B_LOC):
            nc.gpsimd.dma_start(
                ult_sb[:, :, n:n + 1],
                u_t.ap()[n, SEQ - 1, :].rearrange("(k p) -> p k", p=P)[:, :, None])

        # ---- chain-independent work as filler jobs --------------------------
        wot_sb = const.tile([P, 2, D_OUT], F32, tag="wot")
        bt_sb = const.tile([P, 2, NB], F32, tag="bt")
        wc_sb = work.tile([P, 4, NB], F32, tag="wc")
        w_sb = work.tile([B_LOC, NB], F32, tag="w")
        d_sb = work.tile([NB, D_OUT], F32, tag="d")
        bb1_sb = work.tile([NB, 2], F32, tag="bb")  # bb1 duplicated in 2 cols
        wbt_sb = work.tile([NB, B_LOC], F32, tag="wbt")

        S = nc.scalar

        def tr_job(dst, dst_sl, src, src_sl, np_, idp):
            def go():
                tps = psum.tile([np_, idp], F32, tag="sm_tr", bufs=3)
                nc.tensor.transpose(tps[:], src[src_sl], id_sb[0:idp, 0:idp])
                V.tensor_copy(dst[dst_sl], tps[:])
            return go

        def wc_job(mo):
            # WC = W_in^T @ C[:, :NB]   [512, NB], mo-block
            def go():
                ps = psum.tile([P, NB], F32, tag="sm_tr", bufs=3)
                for ko in range(2):
                    MM(ps[:], wi_sb[:, ko, P * mo:P * (mo + 1)],
                       c_sb[:, ko, 0:NB], start=(ko == 0), stop=(ko == 1))
                V.tensor_copy(wc_sb[:, mo, :], ps[:])
            return go

        def w_job():
            # w = u_last @ WC            [2, NB]
            ps = psum.tile([B_LOC, NB], F32, tag="sm_st", bufs=1)
            for ko in range(4):
                MM(ps[:], ult_sb[:, ko, :], wc_sb[:, ko, :],
                   start=(ko == 0), stop=(ko == 3))
            V.tensor_copy(w_sb[:], ps[:])

        def bb1_job():
            # bb1 = C[:, :NB]^T @ b_in  [NB, 1], stored twice
            ps = psum.tile([NB, 1], F32, tag="sm_st", bufs=1)
            for ko in range(2):
                MM(ps[:], c_sb[:, ko, 0:NB], bin_sb[:, ko:ko + 1],
                   start=(ko == 0), stop=(ko == 1))
            V.tensor_copy(bb1_sb[:, 0:1], ps[:])
            V.tensor_copy(bb1_sb[:, 1:2], ps[:])

        def wbt_job():
            # wb^T = w^T + bb1 (bias broadcast over the 2 batch cols) [NB, 2]
            tps = psum.tile([NB, B_LOC], F32, tag="sm_tr", bufs=3)
            nc.tensor.transpose(tps[:], w_sb[:], id_sb[0:B_LOC, 0:B_LOC])
            V.tensor_tensor(wbt_sb[:], tps[:], bb1_sb[:],
                            op=mybir.AluOpType.add)

        ps_out = psum.tile([B_LOC, D_OUT], F32, tag="big")

        def bias_seed_job():
            # out-psum starts as 1 x b_out; the final vt@D accumulates on top
            MM(ps_out[:], ones2_sb[:], bout_sb[:], start=True, stop=False)

        def d_job():
            # D = Btop @ W_out^T         [NB, 512]
            ps = psum.tile([NB, D_OUT], F32, tag="big")
            for ko in range(2):
                MM(ps[:], bt_sb[:, ko, :], wot_sb[:, ko, :],
                   start=(ko == 0), stop=(ko == 1))
            V.tensor_copy(d_sb[:], ps[:])

        jobs = [tr_job(bt_sb, np.s_[:, c, :], btop_sb,
                       np.s_[:, P * c:P * (c + 1)], P, NB)
                for c in range(2)]
        jobs += [tr_job(wot_sb, np.s_[:, r, P * c:P * (c + 1)],
                        wo_sb, np.s_[:, c, P * r:P * (r + 1)], P, P)
                 for r in range(2) for c in range(4)]
        jobs += [wc_job(mo) for mo in range(4)]
        jobs += [bb1_job, bias_seed_job, w_job, d_job, wbt_job]

        def emit_jobs(n):
            for _ in range(n):
                if jobs:
                    jobs.pop(0)()

        # ---- 128x128 power chain: XA = [S | Pacc], st = S^T -----------------
        # xast = [S | Pacc | S^T] in one tile; both per-iter matmuls write
        # one PSUM bank so each iteration needs a single copy + sem wait.
        # Iter 1 runs straight off (a00, a00t) with Pacc_1 = A00, so after
        # iters 2..9 apply factors A^2..A^256, Pacc_9 = A^511 and
        # G00 = S_9 @ Pacc_9 = A^512 @ A^511 = A^1023.
        xa_cur = work.tile([NB, 3 * NB], F32, tag="xa")
        ps = psum.tile([NB, 4 * NB], F32, tag="sm_xa")
        MM(ps[:, 0:NB], a00t_sb[:], a00_sb[:], start=True, stop=True)
        MM(ps[:, 2 * NB:3 * NB], a00_sb[:], a00t_sb[:], start=True, stop=True)
        V.tensor_copy(xa_cur[:, 0:NB], ps[:, 0:NB])
        V.tensor_copy(xa_cur[:, 2 * NB:3 * NB], ps[:, 2 * NB:3 * NB])
        V.tensor_copy(xa_cur[:, NB:2 * NB], a00_sb[:])

        for k in range(2, 10):
            xa_new = work.tile([NB, 3 * NB], F32, tag="xa")
            ps = psum.tile([NB, 4 * NB], F32, tag="sm_xa")
            MM(ps[:, 0:2 * NB], xa_cur[:, 2 * NB:3 * NB], xa_cur[:, 0:2 * NB],
               start=True, stop=True)
            MM(ps[:, 2 * NB:3 * NB], xa_cur[:, 0:NB], xa_cur[:, 2 * NB:3 * NB],
               start=True, stop=True)
            V.tensor_copy(xa_new[:], ps[:, 0:3 * NB])
            xa_cur = xa_new
            emit_jobs([1, 2, 2, 3, 3, 3, 3, 2][k - 2])
        emit_jobs(len(jobs))

        # ---- post-chain tail --------------------------------------------
        # v^T = G00^T wb^T = p9^T (s9^T wb^T);  out = v^T^T @ D + bias-seed
        z_sb = work.tile([NB, B_LOC], F32, tag="z")
        ps = psum.tile([NB, B_LOC], F32, tag="sm_st", bufs=1)
        MM(ps[:], xa_cur[:, 0:NB], wbt_sb[:], start=True, stop=True)
        V.tensor_copy(z_sb[:], ps[:])

        vt_sb = work.tile([NB, B_LOC], F32, tag="vt")
        ps = psum.tile([NB, B_LOC], F32, tag="sm_tr", bufs=3)
        MM(ps[:], xa_cur[:, NB:2 * NB], z_sb[:], start=True, stop=True)
        V.tensor_copy(vt_sb[:], ps[:])

        # split the output tail into halves so half-0's copy+DMA overlaps
        # half-1's matmul; the two DMAs go out on different HWDGE queues
        H2 = D_OUT // 2
        out_sb = work.tile([B_LOC, D_OUT], F32, tag="osb")
        MM(ps_out[:, 0:H2], vt_sb[:], d_sb[:, 0:H2], start=False, stop=False)
        V.tensor_copy(out_sb[:, 0:H2], ps_out[:, 0:H2])
        nc.sync.dma_start(out.ap()[:, 0:H2], out_sb[:, 0:H2])
        MM(ps_out[:, H2:D_OUT], vt_sb[:], d_sb[:, H2:D_OUT],
           start=False, stop=True)
        V.tensor_copy(out_sb[:, H2:D_OUT], ps_out[:, H2:D_OUT])
        nc.scalar.dma_start(out.ap()[:, H2:D_OUT], out_sb[:, H2:D_OUT])

    nc.compile()
    return nc


_NC_CACHE = {}


def _get_nc():
    if "nc" not in _NC_CACHE:
        _NC_CACHE["nc"] = _build()
    return _NC_CACHE["nc"]


_EYE = np.eye(P, dtype=np.float32)
_ONES2 = np.ones((1, B_LOC), dtype=np.float32)


def kernel(u_t, W_in, b_in, C, W_out, b_out, A_d, B_d, **run_kwargs):
    nc = _get_nc()
    u_t = np.ascontiguousarray(u_t, dtype=np.float32)
    shared = {
        "W_in": np.ascontiguousarray(W_in, dtype=np.float32),
        "b_in": np.ascontiguousarray(b_in, dtype=np.float32),
        "C": np.ascontiguousarray(C, dtype=np.float32),
        "W_out": np.ascontiguousarray(W_out, dtype=np.float32),
        "b_out": np.ascontiguousarray(b_out, dtype=np.float32),
        "A_d": np.ascontiguousarray(A_d, dtype=np.float32),
        "B_d": np.ascontiguousarray(B_d, dtype=np.float32),
        "eye": _EYE,
        "a00t": np.ascontiguousarray(np.asarray(A_d, dtype=np.float32)[0:NB, 0:NB].T),
        "ones2": _ONES2,
    }
    in_maps = []
    for i in range(N_CORES):
        m = dict(shared)
        m["u_t"] = np.ascontiguousarray(u_t[i * B_LOC:(i + 1) * B_LOC])
        in_maps.append(m)
    res = bass_utils.run_bass_kernel_spmd(
        nc, in_maps, core_ids=list(range(N_CORES)), **run_kwargs)
    out = np.concatenate([res.results[i]["out"] for i in range(N_CORES)], axis=0)
    if run_kwargs:
        return out, res
    return out


# revision 32
# speedup vs baseline: 1.0641x; 1.0424x over previous
"""Trainium2 Bass kernel for nn_DiscreteStateSpaceModel_77077483094247.

Math: the reference computes y = einsum('nij,ijk->nik', u, K) but only uses
y[:, -1, :], so the whole model collapses to

    out = (u_t[:,-1,:] @ W_in.T + b_in) @ (C @ A_d^1023 @ B_d) @ W_out.T + b_out

A_d^1023 is built with binary exponentiation (9 squarings + 9 multiplies)
instead of the 1023-step serial scan.  Moreover A_d = expm(-0.01*HiPPO) is
lower triangular, so G = A_d^1023 is lower triangular with
G00 = (A_d[:128,:128])^1023 exactly; the coupling block G10 has norm ~2e-11
vs 5e-5 for G00 (validated on the actual inputs: dropping it moves the
output by <1e-7 of output scale), so the chain runs on 128x128 blocks and

    out = (w + 1 x bb1^T) @ G00 @ D + 1 x b_out
    w   = u_last @ W_in^T @ C[:, :128]          (chain-independent)
    bb1 = C[:, :128]^T @ b_in                   (chain-independent)
    D   = B_d[:128, :] @ W_out^T                (chain-independent)

Sharding: u_t is sharded over batch (2 rows per core); the small matrices are
replicated and the chain is duplicated per core (per the spec hint).

Scheduling: the serial 9-step chain (2 small matmuls + 2 PSUM->SBUF copies
per step) leaves PE bubbles; all chain-independent work (WC = W_in^T C,
w = u @ WC, D, bb1, 11 PE transposes) is emitted as filler jobs between
chain iterations so the PE stays dense and the HAM clock stays warm.  After
G00 only v = wb @ G00, a tiny transpose, and out = v^T^T @ D remain.

matmul computes lhsT.T @ rhs; fp32 everywhere (float32r's 11-bit mantissa
amplifies ~400x through the squaring chain - measured 9e-2 rel err).
"""

import numpy as np
from contextlib import ExitStack

from concourse import bacc, bass, mybir, tile
from concourse import bass_utils

B_SZ, SEQ, D_IN, H_DIM, D_OUT = 16, 1024, 512, 256, 512
N_CORES = 8
B_LOC = B_SZ // N_CORES  # 2 batch rows per core

F32 = mybir.dt.float32
P = 128  # partitions
NB = 128  # chain block size


def _build():
    nc = bacc.Bacc("TRN2", target_bir_lowering=False, debug=False,
                   num_devices=N_CORES)

    u_t = nc.dram_tensor("u_t", [B_LOC, SEQ, D_IN], F32, kind="ExternalInput")
    W_in = nc.dram_tensor("W_in", [H_DIM, D_IN], F32, kind="ExternalInput")
    b_in = nc.dram_tensor("b_in", [H_DIM], F32, kind="ExternalInput")
    C_t = nc.dram_tensor("C", [H_DIM, H_DIM], F32, kind="ExternalInput")
    W_out = nc.dram_tensor("W_out", [D_OUT, H_DIM], F32, kind="ExternalInput")
    b_out = nc.dram_tensor("b_out", [D_OUT], F32, kind="ExternalInput")
    A_d = nc.dram_tensor("A_d", [H_DIM, H_DIM], F32, kind="ExternalInput")
    B_d = nc.dram_tensor("B_d", [H_DIM, H_DIM], F32, kind="ExternalInput")
    eye = nc.dram_tensor("eye", [P, P], F32, kind="ExternalInput")
    A00T = nc.dram_tensor("a00t", [NB, NB], F32, kind="ExternalInput")
    ones2 = nc.dram_tensor("ones2", [1, B_LOC], F32, kind="ExternalInput")
    out = nc.dram_tensor("out", [B_LOC, D_OUT], F32, kind="ExternalOutput")

    with tile.TileContext(nc) as tc, ExitStack() as ctx:
        const = ctx.enter_context(tc.tile_pool(name="const", bufs=1))
        work = ctx.enter_context(tc.tile_pool(name="work", bufs=2))
        psum = ctx.enter_context(
            tc.tile_pool(name="psum", bufs=2, space=bass.MemorySpace.PSUM))

        V = nc.vector
        MM = nc.tensor.matmul

        # ---- loads; the chain needs a00 + eye immediately -------------------
        a00_sb = const.tile([NB, NB], F32, tag="a00")
        nc.sync.dma_start(a00_sb[:], A_d.ap()[0:NB, 0:NB])
        a00t_sb = const.tile([NB, NB], F32, tag="a00t")
        nc.scalar.dma_start(a00t_sb[:], A00T.ap()[:, :])
        id_sb = const.tile([P, P], F32, tag="id")
        nc.scalar.dma_start(id_sb[:], eye.ap()[:, :])
        c_sb = const.tile([P, 2, H_DIM], F32, tag="c")
        nc.scalar.dma_start(c_sb[:], C_t.ap().rearrange("(b p) d -> p b d", p=P))

        btop_sb = const.tile([NB, H_DIM], F32, tag="btop")
        nc.sync.dma_start(btop_sb[:], B_d.ap()[0:NB, :])
        wo_sb = const.tile([P, 4, H_DIM], F32, tag="wo")
        nc.sync.dma_start(wo_sb[:], W_out.ap().rearrange("(b p) d -> p b d", p=P))

        wi_sb = const.tile([P, 2, D_IN], F32, tag="wi")
        nc.scalar.dma_start(wi_sb[:], W_in.ap().rearrange("(b p) d -> p b d", p=P))
        bin_sb = const.tile([P, 2], F32, tag="bin")
        nc.scalar.dma_start(bin_sb[:], b_in.ap().rearrange("(b p) -> p b", p=P))
        bout_sb = const.tile([1, D_OUT], F32, tag="bout")
        nc.scalar.dma_start(bout_sb[:], b_out.ap()[None, :])
        ones2_sb = const.tile([1, B_LOC], F32, tag="ones2")
        nc.scalar.dma_start(ones2_sb[:], ones2.ap()[:, :])

        # u_last^T: [512, 2] column layout -> sbuf [128, 4ko, 2]
        ult_sb = const.tile([P, 4, B_LOC], F32, tag="ult")
        for n in range(B_LOC):
            nc.gpsimd.dma_start(
                ult_sb[:, :, n:n + 1],
                u_t.ap()[n, SEQ - 1, :].rearrange("(k p) -> p k", p=P)[:, :, None])

        # ---- chain-independent work as filler jobs --------------------------
        wot_sb = const.tile([P, 2, D_OUT], F32, tag="wot")
        bt_sb = const.tile([P, 2, NB], F32, tag="bt")
        wc_sb = work.tile([P, 4, NB], F32, tag="wc")
        w_sb = work.tile([B_LOC, NB], F32, tag="w")
        d_sb = work.tile([NB, D_OUT], F32, tag="d")
        bb1_sb = work.tile([NB, 2], F32, tag="bb")  # bb1 duplicated in 2 cols
        wbt_sb = work.tile([NB, B_LOC], F32, tag="wbt")

        S = nc.scalar

        def tr_job(dst, dst_sl, src, src_sl, np_, idp):
            def go():
                tps = psum.tile([np_, idp], F32, tag="sm_tr", bufs=3)
                nc.tensor.transpose(tps[:], src[src_sl], id_sb[0:idp, 0:idp])
                V.tensor_copy(dst[dst_sl], tps[:])
            return go

        def wc_job(mo):
            # WC = W_in^T @ C[:, :NB]   [512, NB], mo-block
            def go():
                ps = psum.tile([P, NB], F32, tag="sm_tr", bufs=3)
                for ko in range(2):
                    MM(ps[:], wi_sb[:, ko, P * mo:P * (mo + 1)],
                       c_sb[:, ko, 0:NB], start=(ko == 0), stop=(ko == 1))
                V.tensor_copy(wc_sb[:, mo, :], ps[:])
            return go

        def w_job():
            # w = u_last @ WC            [2, NB]
            ps = psum.tile([B_LOC, NB], F32, tag="sm_st", bufs=1)
            for ko in range(4):
                MM(ps[:], ult_sb[:, ko, :], wc_sb[:, ko, :],
                   start=(ko == 0), stop=(ko == 3))
            V.tensor_copy(w_sb[:], ps[:])

        def bb1_job():
            # bb1 = C[:, :NB]^T @ b_in  [NB, 1], stored twice
            ps = psum.tile([NB, 1], F32, tag="sm_st", bufs=1)
            for ko in range(2):
                MM(ps[:], c_sb[:, ko, 0:NB], bin_sb[:, ko:ko + 1],
                   start=(ko == 0), stop=(ko == 1))
            V.tensor_copy(bb1_sb[:, 0:1], ps[:])
            V.tensor_copy(bb1_sb[:, 1:2], ps[:])

        def wbt_job():
            # wb^T = w^T + bb1 (bias broadcast over the 2 batch cols) [NB, 2]
            tps = psum.tile([NB, B_LOC], F32, tag="sm_tr", bufs=3)
            nc.tensor.transpose(tps[:], w_sb[:], id_sb[0:B_LOC, 0:B_LOC])
            V.tensor_tensor(wbt_sb[:], tps[:], bb1_sb[:],
                            op=mybir.AluOpType.add)

        H2 = D_OUT // 2
        ps_out0 = psum.tile([B_LOC, H2], F32, tag="big")
        ps_out1 = psum.tile([B_LOC, H2], F32, tag="big")

        def bias_seed_job():
            # out-psum halves start as 1 x b_out; the final vt@D accumulates
            MM(ps_out0[:], ones2_sb[:], bout_sb[:, 0:H2], start=True, stop=False)
            MM(ps_out1[:], ones2_sb[:], bout_sb[:, H2:D_OUT], start=True, stop=False)

        def d_job():
            # D = Btop @ W_out^T         [NB, 512]
            ps = psum.tile([NB, D_OUT], F32, tag="sm_st", bufs=1)
            for ko in range(2):
                MM(ps[:], bt_sb[:, ko, :], wot_sb[:, ko, :],
                   start=(ko == 0), stop=(ko == 1))
            V.tensor_copy(d_sb[:], ps[:])

        jobs = [tr_job(bt_sb, np.s_[:, c, :], btop_sb,
                       np.s_[:, P * c:P * (c + 1)], P, NB)
                for c in range(2)]
        jobs += [tr_job(wot_sb, np.s_[:, r, P * c:P * (c + 1)],
                        wo_sb, np.s_[:, c, P * r:P * (r + 1)], P, P)
                 for r in range(2) for c in range(4)]
        jobs += [wc_job(mo) for mo in range(4)]
        jobs += [bb1_job, bias_seed_job, w_job, d_job, wbt_job]

        def emit_jobs(n):
            for _ in range(n):
                if jobs:
                    jobs.pop(0)()

        # ---- 128x128 power chain: XA = [S | Pacc], st = S^T -----------------
        # xast = [S | Pacc | S^T] in one tile; both per-iter matmuls write
        # one PSUM bank so each iteration needs a single copy + sem wait.
        # Iter 1 runs straight off (a00, a00t) with Pacc_1 = A00, so after
        # iters 2..9 apply factors A^2..A^256, Pacc_9 = A^511 and
        # G00 = S_9 @ Pacc_9 = A^512 @ A^511 = A^1023.
        xa_cur = work.tile([NB, 3 * NB], F32, tag="xa")
        ps = psum.tile([NB, 4 * NB], F32, tag="sm_xa")
        MM(ps[:, 0:NB], a00t_sb[:], a00_sb[:], start=True, stop=True)
        MM(ps[:, 2 * NB:3 * NB], a00_sb[:], a00t_sb[:], start=True, stop=True)
        V.tensor_copy(xa_cur[:, 0:NB], ps[:, 0:NB])
        V.tensor_copy(xa_cur[:, 2 * NB:3 * NB], ps[:, 2 * NB:3 * NB])
        V.tensor_copy(xa_cur[:, NB:2 * NB], a00_sb[:])

        for k in range(2, 10):
            xa_new = work.tile([NB, 3 * NB], F32, tag="xa")
            ps = psum.tile([NB, 4 * NB], F32, tag="sm_xa")
            MM(ps[:, 0:2 * NB], xa_cur[:, 2 * NB:3 * NB], xa_cur[:, 0:2 * NB],
               start=True, stop=True)
            MM(ps[:, 2 * NB:3 * NB], xa_cur[:, 0:NB], xa_cur[:, 2 * NB:3 * NB],
               start=True, stop=True)
            V.tensor_copy(xa_new[:], ps[:, 0:3 * NB])
            xa_cur = xa_new
            emit_jobs([1, 2, 2, 3, 3, 3, 3, 2][k - 2])
        emit_jobs(len(jobs))

        # ---- post-chain tail --------------------------------------------
        # v^T = G00^T wb^T = p9^T (s9^T wb^T);  out = v^T^T @ D + bias-seed
        z_sb = work.tile([NB, B_LOC], F32, tag="z")
        ps = psum.tile([NB, B_LOC], F32, tag="sm_st", bufs=1)
        MM(ps[:], xa_cur[:, 0:NB], wbt_sb[:], start=True, stop=True)
        V.tensor_copy(z_sb[:], ps[:])

        vt_sb = work.tile([NB, B_LOC], F32, tag="vt")
        ps = psum.tile([NB, B_LOC], F32, tag="sm_tr", bufs=3)
        MM(ps[:], xa_cur[:, NB:2 * NB], z_sb[:], start=True, stop=True)
        V.tensor_copy(vt_sb[:], ps[:])

        # split the output tail into halves so half-0's copy+DMA overlaps
        # half-1's matmul; the two DMAs go out on different HWDGE queues
        out_sb = work.tile([B_LOC, D_OUT], F32, tag="osb")
        MM(ps_out0[:], vt_sb[:], d_sb[:, 0:H2], start=False, stop=True)
        V.tensor_copy(out_sb[:, 0:H2], ps_out0[:])
        nc.sync.dma_start(out.ap()[:, 0:H2], out_sb[:, 0:H2])
        MM(ps_out1[:], vt_sb[:], d_sb[:, H2:D_OUT], start=False, stop=True)
        V.tensor_copy(out_sb[:, H2:D_OUT], ps_out1[:])
        nc.scalar.dma_start(out.ap()[:, H2:D_OUT], out_sb[:, H2:D_OUT])

    nc.compile()
    return nc


_NC_CACHE = {}


def _get_nc():
    if "nc" not in _NC_CACHE:
        _NC_CACHE["nc"] = _build()
    return _NC_CACHE["nc"]


_EYE = np.eye(P, dtype=np.float32)
_ONES2 = np.ones((1, B_LOC), dtype=np.float32)


def kernel(u_t, W_in, b_in, C, W_out, b_out, A_d, B_d, **run_kwargs):
    nc = _get_nc()
    u_t = np.ascontiguousarray(u_t, dtype=np.float32)
    shared = {
        "W_in": np.ascontiguousarray(W_in, dtype=np.float32),
        "b_in": np.ascontiguousarray(b_in, dtype=np.float32),
        "C": np.ascontiguousarray(C, dtype=np.float32),
        "W_out": np.ascontiguousarray(W_out, dtype=np.float32),
        "b_out": np.ascontiguousarray(b_out, dtype=np.float32),
        "A_d": np.ascontiguousarray(A_d, dtype=np.float32),
        "B_d": np.ascontiguousarray(B_d, dtype=np.float32),
        "eye": _EYE,
        "a00t": np.ascontiguousarray(np.asarray(A_d, dtype=np.float32)[0:NB, 0:NB].T),
        "ones2": _ONES2,
    }
    in_maps = []
    for i in range(N_CORES):
        m = dict(shared)
        m["u_t"] = np.ascontiguousarray(u_t[i * B_LOC:(i + 1) * B_LOC])
        in_maps.append(m)
    res = bass_utils.run_bass_kernel_spmd(
        nc, in_maps, core_ids=list(range(N_CORES)), **run_kwargs)
    out = np.concatenate([res.results[i]["out"] for i in range(N_CORES)], axis=0)
    if run_kwargs:
        return out, res
    return out
